# revision 6
# baseline (speedup 1.0000x reference)
"""Trainium2 Bass kernel for fused QKV-projection + multi-head attention.

Problem: x[2,2048,1024] @ W_qkv[1024,3072] + b -> split q/k/v -> 16 heads of
dim 64 -> softmax(q k^T / 8) v -> [2,2048,1024].

Sharding (8 cores): data-parallel over batch (2) x tensor-parallel over head
groups (4 heads per core).  Each core computes a disjoint output slice
[2048, 256]; no collectives are needed.

Design notes:
- Matmul operands are fp16 (fp32 PSUM accumulation).  x is pre-transposed and
  pre-cast on the host.
- q and k are both stored pair-packed [128, T] (head 2*pr at partitions 0:64,
  head 2*pr+1 at 64:128).  Scores for the two heads of a pair are computed as
  two K=64 row-tiled matmuls (strips 0-1 and 2-3 of the PE array) that run
  CONCURRENTLY in the array and write separate PSUM banks: scoresT [k, q]
  comes out at ~2x the serial rate.
- scoresT [k, q] layout keeps softmax's denominator on the PE (ones-column
  appended to V: [E^T V | E^T 1] accumulated with the numerator).  exp has no
  max-subtraction: scores are bounded for this problem's scale.
- exp alternates engines per k-block: even kb on ACT (true Exp), odd kb on
  DVE as a Schraudolph bit-trick (one tensor_scalar: u16 = 184.664*s + 15316,
  bitcast to fp16 ~= exp(s/8) within ~2%).  Any constant rounding offset is a
  global scale on e and cancels in softmax.  This halves the ACT load, which
  is otherwise the attention-phase bottleneck.
- The q range is processed in 512-wide chunks so PSUM fits: 2 double-buffered
  score slots (2 banks each) + 2 AV accumulators + 2 projection banks = 8.
- v_proj and the pair-1 q/k projections are issued as fillers INSIDE
  attention(pair 0)'s kb loop so the in-order PE queue has work while waiting
  for exp; pair-1 attention then runs without fillers.
- The kernel returns yT [256, T] (head-major, unnormalized) and den [4, T];
  the host divides and transposes.
"""

import sys

sys.path.insert(0, "/opt/trn_rl_repo")

import numpy as np

import concourse.bacc as bacc
import concourse.bass as bass
import concourse.mybir as mybir
import concourse.tile as tile
from concourse.bass import ts

P = 128
T = 2048
D = 1024
NH = 4          # heads per core
HD = 64         # head dim
TB = T // P     # 16 t-blocks
CB = D // P     # 8 c-blocks
QKV_COLS = 3 * NH * HD  # 768 per core
F32 = mybir.dt.float32
F16 = mybir.dt.float16
I16 = mybir.dt.int16

# Schraudolph exp(s/8) in fp16 bits: u16 = round(1024*log2(e)) + 15360 + c
SCH_MUL = 1024.0 * 0.125 * 1.4426950408889634   # 184.664
SCH_ADD = 15360.0 - 44.5 + 0.5                  # +0.5: trunc-to-floor comp

_CACHED = {}


def build_bass(finalize=True):
    nc = bacc.Bacc()

    xT_d = nc.dram_tensor("xT", [D, T], F16, kind="ExternalInput")
    w_d = nc.dram_tensor("w", [D, QKV_COLS], F16, kind="ExternalInput")
    bqk_d = nc.dram_tensor("bqk", [P, 4], F32, kind="ExternalInput")
    bv_d = nc.dram_tensor("bv", [1, NH * HD], F32, kind="ExternalInput")
    y_d = nc.dram_tensor("y", [2 * P, T], F32, kind="ExternalOutput")
    den_d = nc.dram_tensor("den", [NH, T], F32, kind="ExternalOutput")

    with tile.TileContext(nc) as tc:
        with (
            tc.tile_pool(name="persist", bufs=1) as persist,
            tc.tile_pool(name="small", bufs=2) as small,
            tc.tile_pool(name="ystage", bufs=4) as ystage,
            tc.tile_pool(name="epool", bufs=3) as epool,
            tc.tile_pool(name="ps_s", bufs=1, space="PSUM") as ps_s,
            tc.tile_pool(name="ps_y", bufs=1, space="PSUM") as ps_y,
            tc.tile_pool(name="ps_p", bufs=1, space="PSUM") as ps_p,
        ):
            # kT/qT: [p, t] pair-packed; head 2*pr at partitions 0:64,
            # head 2*pr+1 at 64:128
            kT = [persist.tile([P, T], F16, name=f"kT{i}") for i in range(2)]
            qT = [persist.tile([P, T], F16, name=f"qT{i}") for i in range(2)]
            # V' with ones column per head: [t-part, h, 65], one tile per tb
            vv = [
                persist.tile([P, NH, HD + 1], F16, name=f"vv{tb}")
                for tb in range(TB)
            ]
            for tb in range(TB):
                nc.vector.memset(vv[tb][:, :, HD : HD + 1], 1.0)
            bqk_sb = persist.tile([P, 4], F32)
            bvb = persist.tile([P, NH * HD], F32)

            nc.sync.dma_start(out=bqk_sb[:], in_=bqk_d[:, :])
            nc.gpsimd.dma_start(
                out=bvb[:], in_=bv_d[0:1, :].to_broadcast((P, NH * HD))
            )

            # W split per column group so the first projections' weights land
            # before the whole W transfer completes
            wct = [
                persist.tile([P, CB, P], F16, name=f"wct{i}") for i in range(4)
            ]
            wv = persist.tile([P, CB, NH * HD], F16)
            for i in (0, 2):
                nc.sync.dma_start(
                    out=wct[i][:],
                    in_=w_d[:, ts(i, P)].rearrange("(cb p) col -> p cb col", p=P),
                )
            # one tile + two DMAs per c-block so matmuls start on the first
            # chunk and more DMA queues run in parallel
            xTs = [persist.tile([P, T], F16, name=f"xTs{cb}") for cb in range(CB)]
            for cb in range(CB):
                for hh in range(2):
                    nc.sync.dma_start(
                        out=xTs[cb][ts(hh, 64), :],
                        in_=xT_d[cb * P + hh * 64 : cb * P + (hh + 1) * 64, :],
                    )
            nc.sync.dma_start(
                out=wv[:],
                in_=w_d[:, 2 * NH * HD :].rearrange("(cb p) col -> p cb col", p=P),
            )
            for i in (1, 3):
                nc.sync.dma_start(
                    out=wct[i][:],
                    in_=w_d[:, ts(i, P)].rearrange("(cb p) col -> p cb col", p=P),
                )

            # ---------------- QKV projection --------------------------------
            # ct: 0/1 = q pair 0/1, 2/3 = k pair 0/1.  Generator form so the
            # attention loop can interleave single matmuls as PE fillers.
            def qk_proj_gen(ct, chunks=range(4)):
                dst = qT[ct] if ct < 2 else kT[ct - 2]
                for tc2 in chunks:  # 512-wide t-chunks
                    pqk = ps_p.tile([P, 512], F32, tag=f"P{tc2 % 2}", name="pqk")
                    for cb in range(CB):
                        nc.tensor.matmul(
                            pqk[:],
                            lhsT=wct[ct][:, cb, :],
                            rhs=xTs[cb][:, ts(tc2, 512)],
                            start=(cb == 0),
                            stop=(cb == CB - 1),
                        )
                        yield 1
                    nc.vector.tensor_scalar_add(
                        out=dst[:, ts(tc2, 512)],
                        in0=pqk[:],
                        scalar1=bqk_sb[:, ct : ct + 1],
                    )

            def v_proj(tb):
                pv = ps_p.tile([P, NH * HD], F32, tag=f"P{tb % 2}", name="pv")
                for cb in range(CB):
                    nc.tensor.matmul(
                        pv[:],
                        lhsT=xTs[cb][:, ts(tb, P)],
                        rhs=wv[:, cb, :],
                        start=(cb == 0),
                        stop=(cb == CB - 1),
                    )
                nc.vector.tensor_tensor(
                    out=vv[tb][:, :, 0:HD],
                    in0=pv[:].rearrange("p (a b) -> p a b", a=NH),
                    in1=bvb[:].rearrange("p (a b) -> p a b", a=NH),
                    op=mybir.AluOpType.add,
                )

            def run_gen(g):
                for _ in g:
                    pass

            # ---------------- attention -------------------------------------
            # Per 512-wide q chunk: kb pipeline scores(kb) -> exp(kb) (ACT on
            # even kb, DVE Schraudolph on odd kb) while the PE runs AV(kb-1).
            # `filler_cb(qc, kb)` issues projection work into the PE queue
            # each kb, before AV, to cover the exp wait.  Anything AV(kb-1)
            # depends on must already be fully issued (in-order PE queue).
            def attention(pr, filler_cb=None):
                for qc in range(4):  # 512-wide q chunks
                    pY = [
                        ps_y.tile([HD + 1, 512], F32, tag=f"Y{s}", name=f"pY{s}")
                        for s in range(2)
                    ]

                    def issue_av(kb, eprev):
                        for s in range(2):
                            nc.tensor.matmul(
                                pY[s][:],
                                lhsT=vv[kb][:, 2 * pr + s, :],
                                rhs=eprev[:, ts(s, 512)],
                                start=(kb == 0),
                                stop=(kb == TB - 1),
                            )

                    prev = None
                    for kb in range(TB):
                        pS = ps_s.tile(
                            [P, 2 * 512], F32, tag=f"S{kb % 2}", name="pS"
                        )
                        for s in range(2):
                            nc.tensor.matmul(
                                pS[:, ts(s, 512)],
                                lhsT=kT[pr][ts(s, 64), ts(kb, P)],
                                rhs=qT[pr][ts(s, 64), ts(qc, 512)],
                                start=True,
                                stop=True,
                            )
                        eT = epool.tile([P, 2 * 512], F16, name="eT")
                        if kb % 2 == 0:
                            nc.scalar.activation(
                                out=eT[:],
                                in_=pS[:],
                                func=mybir.ActivationFunctionType.Exp,
                                scale=0.125,
                            )
                        else:
                            nc.vector.tensor_scalar(
                                out=eT[:].bitcast(I16),
                                in0=pS[:],
                                scalar1=SCH_MUL,
                                scalar2=SCH_ADD,
                                op0=mybir.AluOpType.mult,
                                op1=mybir.AluOpType.add,
                            )
                        if filler_cb is not None:
                            filler_cb(qc, kb)
                        if prev is not None:
                            issue_av(kb - 1, prev)
                        prev = eT
                    issue_av(TB - 1, prev)

                    # evacuate: numerator rows to SBUF -> HBM (transposed
                    # layout, host finishes); denominator row likewise
                    for s in range(2):
                        yst = ystage.tile([HD, 512], F32, name="yst")
                        nc.vector.tensor_copy(out=yst[:], in_=pY[s][0:HD, :])
                        nc.sync.dma_start(
                            out=y_d[
                                pr * P + s * HD : pr * P + (s + 1) * HD,
                                ts(qc, 512),
                            ],
                            in_=yst[:],
                        )
                        dsb = small.tile([1, 512], F32)
                        nc.vector.tensor_copy(out=dsb[:], in_=pY[s][HD : HD + 1, :])
                        nc.sync.dma_start(
                            out=den_d[2 * pr + s : 2 * pr + s + 1, ts(qc, 512)],
                            in_=dsb[:],
                        )

            # pair-0 q/k first; v_proj + pair-1 projections fill attention(0).
            # v_proj(kb+1) is issued densely inside qc=0 so vv[j] is always
            # fully in the queue before its AV consumer; qk pair-1 drains
            # fine-grained over qc 1-3.
            run_gen(qk_proj_gen(0))
            run_gen(qk_proj_gen(2))
            v_proj(0)

            def chain():
                yield from qk_proj_gen(1)
                yield from qk_proj_gen(3)

            qkfill = chain()

            def filler0(qc, kb):
                if qc == 0:
                    if kb + 1 < TB:
                        v_proj(kb + 1)
                else:
                    for _ in range(2):
                        if next(qkfill, None) is None:
                            break

            attention(0, filler_cb=filler0)
            run_gen(qkfill)  # anything not drained
            attention(1)

    if finalize:
        nc.finalize()
    return nc


def _shard_inputs(x, W_qkv, b_qkv):
    """Build per-core input maps. Core c: batch c//4, head group c%4."""
    x = np.asarray(x, dtype=np.float32)
    W = np.asarray(W_qkv, dtype=np.float32)
    b = np.asarray(b_qkv, dtype=np.float32)
    bf = np.float16
    xT = [np.ascontiguousarray(x[bi].T.astype(bf)) for bi in range(2)]
    in_maps = []
    for c in range(8):
        bi, hg = c // 4, c % 4
        cs = hg * 256  # column start within each of q/k/v blocks
        w_core = np.concatenate(
            [
                W[:, cs : cs + 256],
                W[:, D + cs : D + cs + 256],
                W[:, 2 * D + cs : 2 * D + cs + 256],
            ],
            axis=1,
        ).astype(bf)
        bqk = np.concatenate([b[cs : cs + 256], b[D + cs : D + cs + 256]])
        bqk = np.ascontiguousarray(bqk.reshape(4, 128).T)
        bv = np.ascontiguousarray(b[2 * D + cs : 2 * D + cs + 256].reshape(1, 256))
        in_maps.append(
            {
                "xT": xT[bi],
                "w": np.ascontiguousarray(w_core),
                "bqk": bqk,
                "bv": bv,
            }
        )
    return in_maps


def kernel(x, W_qkv, b_qkv, trace=False):
    from concourse.bass_utils import run_bass_kernel_spmd

    if "nc" not in _CACHED:
        _CACHED["nc"] = build_bass()
    nc = _CACHED["nc"]

    in_maps = _shard_inputs(x, W_qkv, b_qkv)
    res = run_bass_kernel_spmd(nc, in_maps, list(range(8)), trace=trace)
    _CACHED["last_result"] = res

    out = np.empty((2, T, D), dtype=np.float32)
    for c in range(8):
        bi, hg = c // 4, c % 4
        yT = res.results[c]["y"]  # [256, T] unnormalized, head-major
        den = res.results[c]["den"]  # [4, T]
        y = (yT.reshape(NH, HD, T) / den[:, None, :]).transpose(2, 0, 1)
        out[bi, :, hg * 256 : (hg + 1) * 256] = y.reshape(T, NH * HD)
    return out


if __name__ == "__main__":
    nc = build_bass()
    print("built ok")


# revision 12
# speedup vs baseline: 1.1561x; 1.1561x over previous
"""Trainium2 Bass kernel for fused QKV-projection + multi-head attention.

Problem: x[2,2048,1024] @ W_qkv[1024,3072] + b -> split q/k/v -> 16 heads of
dim 64 -> softmax(q k^T / 8) v -> [2,2048,1024].

Sharding (8 cores): data-parallel over batch (2) x tensor-parallel over head
groups (4 heads per core).  Each core computes a disjoint output slice
[2048, 256]; no collectives are needed.

Design notes:
- Matmul operands are fp16 (fp32 PSUM accumulation).  x is pre-transposed and
  pre-cast on the host.
- q and k are both stored pair-packed [128, T] (head 2*pr at partitions 0:64,
  head 2*pr+1 at 64:128).  Scores for the two heads of a pair are computed as
  two K=64 row-tiled matmuls (strips 0-1 and 2-3 of the PE array) that run
  CONCURRENTLY in the array and write separate PSUM banks: scoresT [k, q]
  comes out at ~2x the serial rate.
- scoresT [k, q] layout keeps softmax's denominator on the PE (ones-column
  appended to V: [E^T V | E^T 1] accumulated with the numerator).  exp has no
  max-subtraction: scores are bounded for this problem's scale.
- exp alternates engines per k-block: even kb on ACT (true Exp), odd kb on
  DVE as a Schraudolph bit-trick (one tensor_scalar: u16 = 184.664*s + 15316,
  bitcast to fp16 ~= exp(s/8) within ~2%).  Any constant rounding offset is a
  global scale on e and cancels in softmax.  This halves the ACT load, which
  is otherwise the attention-phase bottleneck.
- The q range is processed in 512-wide chunks so PSUM fits: 2 double-buffered
  score slots (2 banks each) + 2 AV accumulators + 2 projection banks = 8.
- v_proj and the pair-1 q/k projections are issued as fillers INSIDE
  attention(pair 0)'s kb loop so the in-order PE queue has work while waiting
  for exp; pair-1 attention then runs without fillers.
- The kernel returns yT [256, T] (head-major, unnormalized) and den [4, T];
  the host divides and transposes.
"""

import sys

sys.path.insert(0, "/opt/trn_rl_repo")

import numpy as np

import concourse.bacc as bacc
import concourse.bass as bass
import concourse.mybir as mybir
import concourse.tile as tile
from concourse.bass import ts

P = 128
T = 2048
D = 1024
NH = 4          # heads per core
HD = 64         # head dim
TB = T // P     # 16 t-blocks
CB = D // P     # 8 c-blocks
QKV_COLS = 3 * NH * HD  # 768 per core
F32 = mybir.dt.float32
F16 = mybir.dt.float16
I16 = mybir.dt.int16

# Schraudolph exp(s/8) in fp16 bits: u16 = round(1024*log2(e)) + 15360 + c
SCH_MUL = 1024.0 * 0.125 * 1.4426950408889634   # 184.664
SCH_ADD = 15360.0 - 44.5 + 0.5                  # +0.5: trunc-to-floor comp

_CACHED = {}


def build_bass(finalize=True):
    nc = bacc.Bacc()

    xT_d = nc.dram_tensor("xT", [D, T], F16, kind="ExternalInput")
    w_d = nc.dram_tensor("w", [D, QKV_COLS], F16, kind="ExternalInput")
    bqk_d = nc.dram_tensor("bqk", [P, 4], F32, kind="ExternalInput")
    bv_d = nc.dram_tensor("bv", [1, NH * HD], F32, kind="ExternalInput")
    y_d = nc.dram_tensor("y", [2 * P, T], F32, kind="ExternalOutput")
    den_d = nc.dram_tensor("den", [NH, T], F32, kind="ExternalOutput")

    with tile.TileContext(nc) as tc:
        with (
            tc.tile_pool(name="persist", bufs=1) as persist,
            tc.tile_pool(name="small", bufs=2) as small,
            tc.tile_pool(name="ystage", bufs=4) as ystage,
            tc.tile_pool(name="epool", bufs=3) as epool,
            tc.tile_pool(name="ps_s", bufs=1, space="PSUM") as ps_s,
            tc.tile_pool(name="ps_y", bufs=1, space="PSUM") as ps_y,
            tc.tile_pool(name="ps_p", bufs=1, space="PSUM") as ps_p,
        ):
            # kT/qT: [p, t] pair-packed; head 2*pr at partitions 0:64,
            # head 2*pr+1 at 64:128
            kT = [persist.tile([P, T], F16, name=f"kT{i}") for i in range(2)]
            qT = [persist.tile([P, T], F16, name=f"qT{i}") for i in range(2)]
            # V' with ones column per head: [t-part, h, 65], one tile per tb
            vv = [
                persist.tile([P, NH, HD + 1], F16, name=f"vv{tb}")
                for tb in range(TB)
            ]
            for tb in range(TB):
                nc.vector.memset(vv[tb][:, :, HD : HD + 1], 1.0)
            bqk_sb = persist.tile([P, 4], F32)
            bvb = persist.tile([P, NH * HD], F32)

            nc.sync.dma_start(out=bqk_sb[:], in_=bqk_d[:, :])
            nc.gpsimd.dma_start(
                out=bvb[:], in_=bv_d[0:1, :].to_broadcast((P, NH * HD))
            )

            # W split per column group so the first projections' weights land
            # before the whole W transfer completes
            wct = [
                persist.tile([P, CB, P], F16, name=f"wct{i}") for i in range(4)
            ]
            wv = persist.tile([P, CB, NH * HD], F16)
            for i in (0, 2):
                nc.sync.dma_start(
                    out=wct[i][:],
                    in_=w_d[:, ts(i, P)].rearrange("(cb p) col -> p cb col", p=P),
                )
            # x lands t-first-half for all c-blocks, then second half, so the
            # first projection chunks can start before the whole x transfer
            # completes; [64, 1024] pieces keep 2KB DMA lines
            xTs = [persist.tile([P, T], F16, name=f"xTs{cb}") for cb in range(CB)]
            for th in range(2):
                for cb in range(CB):
                    for hh in range(2):
                        nc.sync.dma_start(
                            out=xTs[cb][ts(hh, 64), ts(th, 1024)],
                            in_=xT_d[
                                cb * P + hh * 64 : cb * P + (hh + 1) * 64,
                                ts(th, 1024),
                            ],
                        )
            nc.sync.dma_start(
                out=wv[:],
                in_=w_d[:, 2 * NH * HD :].rearrange("(cb p) col -> p cb col", p=P),
            )
            for i in (1, 3):
                nc.sync.dma_start(
                    out=wct[i][:],
                    in_=w_d[:, ts(i, P)].rearrange("(cb p) col -> p cb col", p=P),
                )

            # ---------------- QKV projection --------------------------------
            # ct: 0/1 = q pair 0/1, 2/3 = k pair 0/1.  Generator form so the
            # attention loop can interleave single matmuls as PE fillers.
            def qk_proj_gen(ct, chunks=range(4)):
                dst = qT[ct] if ct < 2 else kT[ct - 2]
                for tc2 in chunks:  # 512-wide t-chunks
                    pqk = ps_p.tile([P, 512], F32, tag=f"P{tc2 % 2}", name="pqk")
                    for cb in range(CB):
                        nc.tensor.matmul(
                            pqk[:],
                            lhsT=wct[ct][:, cb, :],
                            rhs=xTs[cb][:, ts(tc2, 512)],
                            start=(cb == 0),
                            stop=(cb == CB - 1),
                        )
                        yield 1
                    # bias add: alternate engines so neither exp engine
                    # becomes the bottleneck when these run as fillers
                    if tc2 % 2 == 0:
                        nc.vector.tensor_scalar_add(
                            out=dst[:, ts(tc2, 512)],
                            in0=pqk[:],
                            scalar1=bqk_sb[:, ct : ct + 1],
                        )
                    else:
                        nc.scalar.add(
                            out=dst[:, ts(tc2, 512)],
                            in_=pqk[:],
                            add=bqk_sb[:, ct : ct + 1],
                        )

            def v_proj(tb):
                pv = ps_p.tile([P, NH * HD], F32, tag=f"P{tb % 2}", name="pv")
                for cb in range(CB):
                    nc.tensor.matmul(
                        pv[:],
                        lhsT=xTs[cb][:, ts(tb, P)],
                        rhs=wv[:, cb, :],
                        start=(cb == 0),
                        stop=(cb == CB - 1),
                    )
                nc.vector.tensor_tensor(
                    out=vv[tb][:, :, 0:HD],
                    in0=pv[:].rearrange("p (a b) -> p a b", a=NH),
                    in1=bvb[:].rearrange("p (a b) -> p a b", a=NH),
                    op=mybir.AluOpType.add,
                )

            def run_gen(g):
                for _ in g:
                    pass

            # ---------------- attention -------------------------------------
            # Per 512-wide q chunk: kb pipeline scores(kb) -> exp(kb) (ACT on
            # even kb, DVE Schraudolph on odd kb) while the PE runs AV(kb-2).
            # The 2-block lag gives exp ~2 PE iterations of slack so the
            # in-order PE queue never stalls on it, and lets the two exp
            # engines run concurrently.  `filler_cb(qc, kb)` issues projection
            # work into the PE queue each kb, before AV; anything AV depends
            # on must already be fully issued (in-order PE queue).
            # pY evacuation is deferred into the next chunk's prologue (the
            # next chunk's first AV only comes at kb=2, so the copies overlap
            # its scores/exp instead of stalling the PE).
            pending_evac = [None]

            def attention(pr, filler_cb=None):
                for qc in range(4):  # 512-wide q chunks
                    pY = [
                        ps_y.tile([HD + 1, 512], F32, tag=f"Y{s}", name=f"pY{s}")
                        for s in range(2)
                    ]

                    def issue_av(kb, eprev, pY=pY, pr=pr):
                        for s in range(2):
                            nc.tensor.matmul(
                                pY[s][:],
                                lhsT=vv[kb][:, 2 * pr + s, :],
                                rhs=eprev[:, ts(s, 512)],
                                start=(kb == 0),
                                stop=(kb == TB - 1),
                            )

                    def evac(pY=pY, pr=pr, qc=qc):
                        # one [65,512] copy per head (numerator + den row),
                        # split across ACT and DVE; two DMAs out of it
                        for s in range(2):
                            yst = ystage.tile([HD + 1, 512], F32, name="yst")
                            if s == 0:
                                nc.scalar.copy(out=yst[:], in_=pY[s][:])
                            else:
                                nc.vector.tensor_copy(out=yst[:], in_=pY[s][:])
                            nc.sync.dma_start(
                                out=y_d[
                                    pr * P + s * HD : pr * P + (s + 1) * HD,
                                    ts(qc, 512),
                                ],
                                in_=yst[0:HD, :],
                            )
                            nc.sync.dma_start(
                                out=den_d[
                                    2 * pr + s : 2 * pr + s + 1, ts(qc, 512)
                                ],
                                in_=yst[HD : HD + 1, :],
                            )

                    e_hist = []
                    for kb in range(TB):
                        pS = ps_s.tile(
                            [P, 2 * 512], F32, tag=f"S{kb % 2}", name="pS"
                        )
                        for s in range(2):
                            nc.tensor.matmul(
                                pS[:, ts(s, 512)],
                                lhsT=kT[pr][ts(s, 64), ts(kb, P)],
                                rhs=qT[pr][ts(s, 64), ts(qc, 512)],
                                start=True,
                                stop=True,
                            )
                        eT = epool.tile([P, 2 * 512], F16, name="eT")
                        if kb % 2 == 0:
                            nc.scalar.activation(
                                out=eT[:],
                                in_=pS[:],
                                func=mybir.ActivationFunctionType.Exp,
                                scale=0.125,
                            )
                        else:
                            nc.vector.tensor_scalar(
                                out=eT[:].bitcast(I16),
                                in0=pS[:],
                                scalar1=SCH_MUL,
                                scalar2=SCH_ADD,
                                op0=mybir.AluOpType.mult,
                                op1=mybir.AluOpType.add,
                            )
                        e_hist.append(eT)
                        if kb < 2 and pending_evac[0] is not None:
                            pending_evac[0][kb]()
                            if kb == 1:
                                pending_evac[0] = None
                        if filler_cb is not None:
                            filler_cb(qc, kb)
                        if kb >= 2:
                            issue_av(kb - 2, e_hist[kb - 2])
                    issue_av(TB - 2, e_hist[TB - 2])
                    # the last AV + evacuation are deferred into the next
                    # chunk's prologue (kb=0 and kb=1) so they never stall
                    pending_evac[0] = (
                        lambda eh=e_hist, ia=issue_av: ia(TB - 1, eh[TB - 1]),
                        evac,
                    )

            # pair-0 q/k first; v_proj + pair-1 projections fill attention(0).
            # v_proj(kb+1) is issued densely inside qc=0 so vv[j] is always
            # fully in the queue before its AV consumer; qk pair-1 drains
            # fine-grained over qc 1-3.
            run_gen(qk_proj_gen(0))
            run_gen(qk_proj_gen(2))
            v_proj(0)

            def chain():
                yield from qk_proj_gen(1)
                yield from qk_proj_gen(3)

            qkfill = chain()

            def filler0(qc, kb):
                if qc == 0:
                    if kb + 1 < TB:
                        v_proj(kb + 1)
                else:
                    for _ in range(2):
                        if next(qkfill, None) is None:
                            break

            attention(0, filler_cb=filler0)
            run_gen(qkfill)  # anything not drained
            attention(1)
            pending_evac[0][0]()  # flush the last chunk's final AV
            pending_evac[0][1]()  # and its evacuation

    if finalize:
        nc.finalize()
    return nc


def _shard_inputs(x, W_qkv, b_qkv):
    """Build per-core input maps. Core c: batch c//4, head group c%4."""
    x = np.asarray(x, dtype=np.float32)
    W = np.asarray(W_qkv, dtype=np.float32)
    b = np.asarray(b_qkv, dtype=np.float32)
    bf = np.float16
    xT = [np.ascontiguousarray(x[bi].T.astype(bf)) for bi in range(2)]
    in_maps = []
    for c in range(8):
        bi, hg = c // 4, c % 4
        cs = hg * 256  # column start within each of q/k/v blocks
        w_core = np.concatenate(
            [
                W[:, cs : cs + 256],
                W[:, D + cs : D + cs + 256],
                W[:, 2 * D + cs : 2 * D + cs + 256],
            ],
            axis=1,
        ).astype(bf)
        bqk = np.concatenate([b[cs : cs + 256], b[D + cs : D + cs + 256]])
        bqk = np.ascontiguousarray(bqk.reshape(4, 128).T)
        bv = np.ascontiguousarray(b[2 * D + cs : 2 * D + cs + 256].reshape(1, 256))
        in_maps.append(
            {
                "xT": xT[bi],
                "w": np.ascontiguousarray(w_core),
                "bqk": bqk,
                "bv": bv,
            }
        )
    return in_maps


def kernel(x, W_qkv, b_qkv, trace=False):
    from concourse.bass_utils import run_bass_kernel_spmd

    if "nc" not in _CACHED:
        _CACHED["nc"] = build_bass()
    nc = _CACHED["nc"]

    in_maps = _shard_inputs(x, W_qkv, b_qkv)
    res = run_bass_kernel_spmd(nc, in_maps, list(range(8)), trace=trace)
    _CACHED["last_result"] = res

    out = np.empty((2, T, D), dtype=np.float32)
    for c in range(8):
        bi, hg = c // 4, c % 4
        yT = res.results[c]["y"]  # [256, T] unnormalized, head-major
        den = res.results[c]["den"]  # [4, T]
        y = (yT.reshape(NH, HD, T) / den[:, None, :]).transpose(2, 0, 1)
        out[bi, :, hg * 256 : (hg + 1) * 256] = y.reshape(T, NH * HD)
    return out


if __name__ == "__main__":
    nc = build_bass()
    print("built ok")


# revision 18
# speedup vs baseline: 1.1671x; 1.0095x over previous
"""Trainium2 Bass kernel for fused QKV-projection + multi-head attention.

Problem: x[2,2048,1024] @ W_qkv[1024,3072] + b -> split q/k/v -> 16 heads of
dim 64 -> softmax(q k^T / 8) v -> [2,2048,1024].

Sharding (8 cores): data-parallel over batch (2) x tensor-parallel over head
groups (4 heads per core).  Each core computes a disjoint output slice
[2048, 256]; no collectives are needed.

Design notes:
- Matmul operands are fp16 (fp32 PSUM accumulation).  x is pre-transposed and
  pre-cast on the host.
- q and k are both stored pair-packed [128, T] (head 2*pr at partitions 0:64,
  head 2*pr+1 at 64:128).  Scores for the two heads of a pair are computed as
  two K=64 row-tiled matmuls (strips 0-1 and 2-3 of the PE array) that run
  CONCURRENTLY in the array and write separate PSUM banks: scoresT [k, q]
  comes out at ~2x the serial rate.
- scoresT [k, q] layout keeps softmax's denominator on the PE (ones-column
  appended to V: [E^T V | E^T 1] accumulated with the numerator).  exp has no
  max-subtraction: scores are bounded for this problem's scale.
- exp alternates engines per k-block: even kb on ACT (true Exp), odd kb on
  DVE as a Schraudolph bit-trick (one tensor_scalar: u16 = 184.664*s + 15316,
  bitcast to fp16 ~= exp(s/8) within ~2%).  Any constant rounding offset is a
  global scale on e and cancels in softmax.  This halves the ACT load, which
  is otherwise the attention-phase bottleneck.
- The q range is processed in 512-wide chunks so PSUM fits: 2 double-buffered
  score slots (2 banks each) + 2 AV accumulators + 2 projection banks = 8.
- v_proj and the pair-1 q/k projections are issued as fillers INSIDE
  attention(pair 0)'s kb loop so the in-order PE queue has work while waiting
  for exp; pair-1 attention then runs without fillers.
- The kernel returns yT [256, T] (head-major, unnormalized) and den [4, T];
  the host divides and transposes.
"""

import sys

sys.path.insert(0, "/opt/trn_rl_repo")

import numpy as np

import concourse.bacc as bacc
import concourse.bass as bass
import concourse.mybir as mybir
import concourse.tile as tile
from concourse.bass import ts

P = 128
T = 2048
D = 1024
NH = 4          # heads per core
HD = 64         # head dim
TB = T // P     # 16 t-blocks
CB = D // P     # 8 c-blocks
QKV_COLS = 3 * NH * HD  # 768 per core
F32 = mybir.dt.float32
F16 = mybir.dt.float16
I16 = mybir.dt.int16

# Schraudolph exp(s/8) in fp16 bits: u16 = round(1024*log2(e)) + 15360 + c
SCH_MUL = 1024.0 * 0.125 * 1.4426950408889634   # 184.664
SCH_ADD = 15360.0 - 44.5 + 0.5                  # +0.5: trunc-to-floor comp

_CACHED = {}


def build_bass(finalize=True):
    nc = bacc.Bacc()

    xT_d = nc.dram_tensor("xT", [D, T], F16, kind="ExternalInput")
    w_d = nc.dram_tensor("w", [D, QKV_COLS], F16, kind="ExternalInput")
    bqk_d = nc.dram_tensor("bqk", [P, 4], F32, kind="ExternalInput")
    bv_d = nc.dram_tensor("bv", [1, NH * HD], F32, kind="ExternalInput")
    y_d = nc.dram_tensor("y", [2 * P, T], F32, kind="ExternalOutput")
    den_d = nc.dram_tensor("den", [NH, T], F32, kind="ExternalOutput")

    with tile.TileContext(nc) as tc:
        with (
            tc.tile_pool(name="persist", bufs=1) as persist,
            tc.tile_pool(name="small", bufs=2) as small,
            tc.tile_pool(name="ystage", bufs=4) as ystage,
            tc.tile_pool(name="epool", bufs=4) as epool,
            tc.tile_pool(name="ps_s", bufs=1, space="PSUM") as ps_s,
            tc.tile_pool(name="ps_y", bufs=1, space="PSUM") as ps_y,
            tc.tile_pool(name="ps_p", bufs=1, space="PSUM") as ps_p,
        ):
            # kT/qT: [p, t] pair-packed; head 2*pr at partitions 0:64,
            # head 2*pr+1 at 64:128
            kT = [persist.tile([P, T], F16, name=f"kT{i}") for i in range(2)]
            qT = [persist.tile([P, T], F16, name=f"qT{i}") for i in range(2)]
            # V' with ones column per head, zero-padded to 128 weight columns
            # so the AV lhsT is a full [128,128] load (FWL-eligible, and the
            # matmul runs at M=128 for the same N cycles; rows 65:128 of the
            # output are garbage and never read): [t-part, h, 128], one per tb
            vv = [
                persist.tile([P, NH, P], F16, name=f"vv{tb}")
                for tb in range(TB)
            ]
            for tb in range(TB):
                nc.vector.memset(vv[tb][:, :, HD:], 0.0)
                nc.vector.memset(vv[tb][:, :, HD : HD + 1], 1.0)
            bqk_sb = persist.tile([P, 4], F32)
            bvb = persist.tile([P, NH * HD], F32)

            nc.sync.dma_start(out=bqk_sb[:], in_=bqk_d[:, :])
            nc.gpsimd.dma_start(
                out=bvb[:], in_=bv_d[0:1, :].to_broadcast((P, NH * HD))
            )

            # W split per column group so the first projections' weights land
            # before the whole W transfer completes
            wct = [
                persist.tile([P, CB, P], F16, name=f"wct{i}") for i in range(4)
            ]
            wv = persist.tile([P, CB, NH * HD], F16)
            for i in (0, 2):
                nc.sync.dma_start(
                    out=wct[i][:],
                    in_=w_d[:, ts(i, P)].rearrange("(cb p) col -> p cb col", p=P),
                )
            # x lands t-first-half for all c-blocks, then second half, so the
            # first projection chunks can start before the whole x transfer
            # completes; [64, 1024] pieces keep 2KB DMA lines
            xTs = [persist.tile([P, T], F16, name=f"xTs{cb}") for cb in range(CB)]
            for th in range(2):
                for cb in range(CB):
                    for hh in range(2):
                        nc.sync.dma_start(
                            out=xTs[cb][ts(hh, 64), ts(th, 1024)],
                            in_=xT_d[
                                cb * P + hh * 64 : cb * P + (hh + 1) * 64,
                                ts(th, 1024),
                            ],
                        )
            nc.sync.dma_start(
                out=wv[:],
                in_=w_d[:, 2 * NH * HD :].rearrange("(cb p) col -> p cb col", p=P),
            )
            for i in (1, 3):
                nc.sync.dma_start(
                    out=wct[i][:],
                    in_=w_d[:, ts(i, P)].rearrange("(cb p) col -> p cb col", p=P),
                )

            # ---------------- QKV projection --------------------------------
            # ct: 0/1 = q pair 0/1, 2/3 = k pair 0/1.  Generator form so the
            # attention loop can interleave single matmuls as PE fillers.
            def qk_proj_gen(ct, chunks=(0, 2)):
                # two 512-wide t-chunks per weight load (accumulating into
                # both P banks) so each wct LDWEIGHTS covers 2 matmuls
                dst = qT[ct] if ct < 2 else kT[ct - 2]
                for tc2 in chunks:
                    pqk = [
                        ps_p.tile([P, 512], F32, tag=f"P{h}", name="pqk")
                        for h in range(2)
                    ]
                    for cb in range(CB):
                        for h in range(2):
                            nc.tensor.matmul(
                                pqk[h][:],
                                lhsT=wct[ct][:, cb, :],
                                rhs=xTs[cb][:, ts(tc2 + h, 512)],
                                start=(cb == 0),
                                stop=(cb == CB - 1),
                            )
                        yield 1
                    # bias add: alternate engines so neither exp engine
                    # becomes the bottleneck when these run as fillers
                    for h in range(2):
                        if h == 0:
                            nc.vector.tensor_scalar_add(
                                out=dst[:, ts(tc2 + h, 512)],
                                in0=pqk[h][:],
                                scalar1=bqk_sb[:, ct : ct + 1],
                            )
                        else:
                            nc.scalar.add(
                                out=dst[:, ts(tc2 + h, 512)],
                                in_=pqk[h][:],
                                add=bqk_sb[:, ct : ct + 1],
                            )

            def v_proj(tb):
                pv = ps_p.tile([P, NH * HD], F32, tag=f"P{tb % 2}", name="pv")
                for cb in range(CB):
                    nc.tensor.matmul(
                        pv[:],
                        lhsT=xTs[cb][:, ts(tb, P)],
                        rhs=wv[:, cb, :],
                        start=(cb == 0),
                        stop=(cb == CB - 1),
                    )
                nc.vector.tensor_tensor(
                    out=vv[tb][:, :, 0:HD],
                    in0=pv[:].rearrange("p (a b) -> p a b", a=NH),
                    in1=bvb[:].rearrange("p (a b) -> p a b", a=NH),
                    op=mybir.AluOpType.add,
                )

            def run_gen(g):
                for _ in g:
                    pass

            # ---------------- attention -------------------------------------
            # Per 512-wide q chunk: kb pipeline scores(kb) -> exp(kb) (ACT on
            # even kb, DVE Schraudolph on odd kb) while the PE runs AV(kb-2).
            # The 2-block lag gives exp ~2 PE iterations of slack so the
            # in-order PE queue never stalls on it, and lets the two exp
            # engines run concurrently.  `filler_cb(qc, kb)` issues projection
            # work into the PE queue each kb, before AV; anything AV depends
            # on must already be fully issued (in-order PE queue).
            # pY evacuation is deferred into the next chunk's prologue (the
            # next chunk's first AV only comes at kb=2, so the copies overlap
            # its scores/exp instead of stalling the PE).
            pending_evac = [None]

            def attention(pr, filler_cb=None):
                for qc in range(4):  # 512-wide q chunks
                    pY = [
                        ps_y.tile([P, 512], F32, tag=f"Y{s}", name=f"pY{s}")
                        for s in range(2)
                    ]

                    def issue_av(kb, eprev, pY=pY, pr=pr):
                        for s in range(2):
                            nc.tensor.matmul(
                                pY[s][:],
                                lhsT=vv[kb][:, 2 * pr + s, :],
                                rhs=eprev[:, ts(s, 512)],
                                start=(kb == 0),
                                stop=(kb == TB - 1),
                            )

                    def evac(pY=pY, pr=pr, qc=qc):
                        # one [65,512] copy per head (numerator + den row),
                        # split across ACT and DVE; two DMAs out of it
                        for s in range(2):
                            yst = ystage.tile([HD + 1, 512], F32, name="yst")
                            if s == 0:
                                nc.scalar.copy(out=yst[:], in_=pY[s][0 : HD + 1, :])
                            else:
                                nc.vector.tensor_copy(
                                    out=yst[:], in_=pY[s][0 : HD + 1, :]
                                )
                            nc.sync.dma_start(
                                out=y_d[
                                    pr * P + s * HD : pr * P + (s + 1) * HD,
                                    ts(qc, 512),
                                ],
                                in_=yst[0:HD, :],
                            )
                            nc.sync.dma_start(
                                out=den_d[
                                    2 * pr + s : 2 * pr + s + 1, ts(qc, 512)
                                ],
                                in_=yst[HD : HD + 1, :],
                            )

                    # kb handled in batches of 2: scores+exp for (2j, 2j+1),
                    # then AVs for (2j-2, 2j-1).  Grouping the two scores
                    # pairs (and the four AVs) lets each group's weight loads
                    # hide under the other pair's matmuls, and the one-batch
                    # AV lag gives each exp ~2us before its AV consumer.
                    e_hist = []
                    for j in range(TB // 2):
                        for kb in (2 * j, 2 * j + 1):
                            pS = ps_s.tile(
                                [P, 2 * 512], F32, tag=f"S{kb % 2}", name="pS"
                            )
                            for s in range(2):
                                nc.tensor.matmul(
                                    pS[:, ts(s, 512)],
                                    lhsT=kT[pr][ts(s, 64), ts(kb, P)],
                                    rhs=qT[pr][ts(s, 64), ts(qc, 512)],
                                    start=True,
                                    stop=True,
                                )
                            eT = epool.tile([P, 2 * 512], F16, name="eT")
                            if kb % 2 == 0:
                                nc.scalar.activation(
                                    out=eT[:],
                                    in_=pS[:],
                                    func=mybir.ActivationFunctionType.Exp,
                                    scale=0.125,
                                )
                            else:
                                nc.vector.tensor_scalar(
                                    out=eT[:].bitcast(I16),
                                    in0=pS[:],
                                    scalar1=SCH_MUL,
                                    scalar2=SCH_ADD,
                                    op0=mybir.AluOpType.mult,
                                    op1=mybir.AluOpType.add,
                                )
                            e_hist.append(eT)
                            if kb < 2 and pending_evac[0] is not None:
                                pending_evac[0][kb]()
                                if kb == 1:
                                    pending_evac[0] = None
                            if filler_cb is not None:
                                filler_cb(qc, kb)
                        if j >= 1:
                            issue_av(2 * j - 2, e_hist[2 * j - 2])
                            issue_av(2 * j - 1, e_hist[2 * j - 1])
                    issue_av(TB - 2, e_hist[TB - 2])
                    # the last AV + evacuation are deferred into the next
                    # chunk's prologue (kb=0 and kb=1) so they never stall
                    pending_evac[0] = (
                        lambda eh=e_hist, ia=issue_av: ia(TB - 1, eh[TB - 1]),
                        evac,
                    )

            # pair-0 q/k first; v_proj + pair-1 projections fill attention(0).
            # v_proj(kb+1) is issued densely inside qc=0 so vv[j] is always
            # fully in the queue before its AV consumer; qk pair-1 drains
            # fine-grained over qc 1-3.
            run_gen(qk_proj_gen(0))
            run_gen(qk_proj_gen(2))
            v_proj(0)

            def chain():
                yield from qk_proj_gen(1)
                yield from qk_proj_gen(3)

            qkfill = chain()

            def filler0(qc, kb):
                if qc == 0:
                    if kb + 1 < TB:
                        v_proj(kb + 1)
                else:
                    for _ in range(2):
                        if next(qkfill, None) is None:
                            break

            attention(0, filler_cb=filler0)
            run_gen(qkfill)  # anything not drained
            attention(1)
            pending_evac[0][0]()  # flush the last chunk's final AV
            pending_evac[0][1]()  # and its evacuation

    if finalize:
        nc.finalize()
    return nc


def _shard_inputs(x, W_qkv, b_qkv):
    """Build per-core input maps. Core c: batch c//4, head group c%4."""
    x = np.asarray(x, dtype=np.float32)
    W = np.asarray(W_qkv, dtype=np.float32)
    b = np.asarray(b_qkv, dtype=np.float32)
    bf = np.float16
    xT = [np.ascontiguousarray(x[bi].T.astype(bf)) for bi in range(2)]
    in_maps = []
    for c in range(8):
        bi, hg = c // 4, c % 4
        cs = hg * 256  # column start within each of q/k/v blocks
        w_core = np.concatenate(
            [
                W[:, cs : cs + 256],
                W[:, D + cs : D + cs + 256],
                W[:, 2 * D + cs : 2 * D + cs + 256],
            ],
            axis=1,
        ).astype(bf)
        bqk = np.concatenate([b[cs : cs + 256], b[D + cs : D + cs + 256]])
        bqk = np.ascontiguousarray(bqk.reshape(4, 128).T)
        bv = np.ascontiguousarray(b[2 * D + cs : 2 * D + cs + 256].reshape(1, 256))
        in_maps.append(
            {
                "xT": xT[bi],
                "w": np.ascontiguousarray(w_core),
                "bqk": bqk,
                "bv": bv,
            }
        )
    return in_maps


def kernel(x, W_qkv, b_qkv, trace=False):
    from concourse.bass_utils import run_bass_kernel_spmd

    if "nc" not in _CACHED:
        _CACHED["nc"] = build_bass()
    nc = _CACHED["nc"]

    in_maps = _shard_inputs(x, W_qkv, b_qkv)
    res = run_bass_kernel_spmd(nc, in_maps, list(range(8)), trace=trace)
    _CACHED["last_result"] = res

    out = np.empty((2, T, D), dtype=np.float32)
    for c in range(8):
        bi, hg = c // 4, c % 4
        yT = res.results[c]["y"]  # [256, T] unnormalized, head-major
        den = res.results[c]["den"]  # [4, T]
        y = (yT.reshape(NH, HD, T) / den[:, None, :]).transpose(2, 0, 1)
        out[bi, :, hg * 256 : (hg + 1) * 256] = y.reshape(T, NH * HD)
    return out


if __name__ == "__main__":
    nc = build_bass()
    print("built ok")


# revision 23
# speedup vs baseline: 1.2731x; 1.0908x over previous
"""Trainium2 Bass kernel for fused QKV-projection + multi-head attention.

Problem: x[2,2048,1024] @ W_qkv[1024,3072] + b -> split q/k/v -> 16 heads of
dim 64 -> softmax(q k^T / 8) v -> [2,2048,1024].

Sharding (8 cores): data-parallel over batch (2) x tensor-parallel over head
groups (4 heads per core).  Each core computes a disjoint output slice
[2048, 256]; no collectives are needed.

Design notes:
- Matmul operands are fp16 (fp32 PSUM accumulation).  x is pre-transposed and
  pre-cast on the host.
- q and k are both stored pair-packed [128, T] (head 2*pr at partitions 0:64,
  head 2*pr+1 at 64:128).  Scores for the two heads of a pair are computed as
  two K=64 row-tiled matmuls (strips 0-1 and 2-3 of the PE array) that run
  CONCURRENTLY in the array and write separate PSUM banks: scoresT [k, q]
  comes out at ~2x the serial rate.
- scoresT [k, q] layout keeps softmax's denominator on the PE (ones-column
  appended to V: [E^T V | E^T 1] accumulated with the numerator).  exp has no
  max-subtraction: scores are bounded for this problem's scale.
- exp alternates engines per k-block: even kb on ACT (true Exp), odd kb on
  DVE as a Schraudolph bit-trick (one tensor_scalar: u16 = 184.664*s + 15316,
  bitcast to fp16 ~= exp(s/8) within ~2%).  Any constant rounding offset is a
  global scale on e and cancels in softmax.  This halves the ACT load, which
  is otherwise the attention-phase bottleneck.
- The q range is processed in 512-wide chunks so PSUM fits: 2 double-buffered
  score slots (2 banks each) + 2 AV accumulators + 2 projection banks = 8.
- v_proj and the pair-1 q/k projections are issued as fillers INSIDE
  attention(pair 0)'s kb loop so the in-order PE queue has work while waiting
  for exp; pair-1 attention then runs without fillers.
- The kernel returns yT [256, T] (head-major, unnormalized) and den [4, T];
  the host divides and transposes.
"""

import sys

sys.path.insert(0, "/opt/trn_rl_repo")

import numpy as np

import concourse.bacc as bacc
import concourse.bass as bass
import concourse.mybir as mybir
import concourse.tile as tile
from concourse.bass import ts

P = 128
T = 2048
D = 1024
NH = 4          # heads per core
HD = 64         # head dim
TB = T // P     # 16 t-blocks
CB = D // P     # 8 c-blocks
QKV_COLS = 3 * NH * HD  # 768 per core
F32 = mybir.dt.float32
F16 = mybir.dt.float16
I16 = mybir.dt.int16

# Schraudolph exp(s/8) in fp16 bits: u16 = round(1024*log2(e)) + 15360 + c
SCH_MUL = 1024.0 * 0.125 * 1.4426950408889634   # 184.664
SCH_ADD = 15360.0 - 44.5 + 0.5                  # +0.5: trunc-to-floor comp

_CACHED = {}


def build_bass(finalize=True):
    nc = bacc.Bacc()

    xT_d = nc.dram_tensor("xT", [D, T], F16, kind="ExternalInput")
    w_d = nc.dram_tensor("w", [D, QKV_COLS], F16, kind="ExternalInput")
    bqk_d = nc.dram_tensor("bqk", [P, 4], F32, kind="ExternalInput")
    bv_d = nc.dram_tensor("bv", [1, NH * HD], F32, kind="ExternalInput")
    y_d = nc.dram_tensor("y", [2 * P, T], F32, kind="ExternalOutput")
    den_d = nc.dram_tensor("den", [NH, T], F32, kind="ExternalOutput")

    with tile.TileContext(nc) as tc:
        with (
            tc.tile_pool(name="persist", bufs=1) as persist,
            tc.tile_pool(name="small", bufs=2) as small,
            tc.tile_pool(name="ystage", bufs=4) as ystage,
            tc.tile_pool(name="epool", bufs=4) as epool,
            tc.tile_pool(name="ps", bufs=1, space="PSUM") as ps,
        ):
            # kT/qT: [p, t] pair-packed; head 2*pr at partitions 0:64,
            # head 2*pr+1 at 64:128
            kT = [persist.tile([P, T], F16, name=f"kT{i}") for i in range(2)]
            qT = [persist.tile([P, T], F16, name=f"qT{i}") for i in range(2)]
            # V' with ones column per head, zero-padded to 128 weight columns
            # so the AV lhsT is a full [128,128] load (FWL-eligible, and the
            # matmul runs at M=128 for the same N cycles; rows 65:128 of the
            # output are garbage and never read): [t-part, h, 128], one per tb
            vv = [
                persist.tile([P, NH, P], F16, name=f"vv{tb}")
                for tb in range(TB)
            ]
            for tb in range(TB):
                nc.vector.memset(vv[tb][:, :, HD:], 0.0)
                nc.vector.memset(vv[tb][:, :, HD : HD + 1], 1.0)
            bqk_sb = persist.tile([P, 4], F32)
            bvb = persist.tile([P, NH * HD], F32)

            nc.sync.dma_start(out=bqk_sb[:], in_=bqk_d[:, :])
            nc.gpsimd.dma_start(
                out=bvb[:], in_=bv_d[0:1, :].to_broadcast((P, NH * HD))
            )

            # W split per column group so the first projections' weights land
            # before the whole W transfer completes
            wct = [
                persist.tile([P, CB, P], F16, name=f"wct{i}") for i in range(4)
            ]
            wv = persist.tile([P, CB, NH * HD], F16)
            # x lands t-first-half for all c-blocks first, so the first
            # projection chunks (and v_proj block 0) can start before the
            # whole x transfer completes; [64, 1024] pieces keep 2KB lines
            xTs = [persist.tile([P, T], F16, name=f"xTs{cb}") for cb in range(CB)]

            def dma_x(th):
                for cb in range(CB):
                    for hh in range(2):
                        nc.sync.dma_start(
                            out=xTs[cb][ts(hh, 64), ts(th, 1024)],
                            in_=xT_d[
                                cb * P + hh * 64 : cb * P + (hh + 1) * 64,
                                ts(th, 1024),
                            ],
                        )

            def dma_w(i):
                nc.sync.dma_start(
                    out=wct[i][:],
                    in_=w_d[:, ts(i, P)].rearrange("(cb p) col -> p cb col", p=P),
                )

            dma_w(2)
            dma_x(0)
            dma_w(0)
            nc.sync.dma_start(
                out=wv[:],
                in_=w_d[:, 2 * NH * HD :].rearrange("(cb p) col -> p cb col", p=P),
            )
            dma_x(1)
            dma_w(1)
            dma_w(3)

            # ---------------- QKV projection --------------------------------
            # ct: 0/1 = q pair 0/1, 2/3 = k pair 0/1.  Two 512-wide t-chunks
            # per weight load (accumulating into two psum banks) so each wct
            # LDWEIGHTS covers 2 matmuls.  Projections run dense (the PE is
            # the binding engine either way); psum borrows the Y tags, which
            # attention only uses later.
            def qk_proj(ct):
                dst = qT[ct] if ct < 2 else kT[ct - 2]
                for pi, tc2 in enumerate((0, 2)):
                    yb = 2 * (pi % 2)
                    pqk = [
                        ps.tile([P, 512], F32, tag=f"Y{yb + h}", name="pqk")
                        for h in range(2)
                    ]
                    for cb in range(CB):
                        for h in range(2):
                            nc.tensor.matmul(
                                pqk[h][:],
                                lhsT=wct[ct][:, cb, :],
                                rhs=xTs[cb][:, ts(tc2 + h, 512)],
                                start=(cb == 0),
                                stop=(cb == CB - 1),
                            )
                    for h in range(2):
                        if h == 0:
                            nc.vector.tensor_scalar_add(
                                out=dst[:, ts(tc2 + h, 512)],
                                in0=pqk[h][:],
                                scalar1=bqk_sb[:, ct : ct + 1],
                            )
                        else:
                            nc.scalar.add(
                                out=dst[:, ts(tc2 + h, 512)],
                                in_=pqk[h][:],
                                add=bqk_sb[:, ct : ct + 1],
                            )

            def v_proj(tb):
                pv = ps.tile([P, NH * HD], F32, tag=f"Y{tb % 4}", name="pv")
                for cb in range(CB):
                    nc.tensor.matmul(
                        pv[:],
                        lhsT=xTs[cb][:, ts(tb, P)],
                        rhs=wv[:, cb, :],
                        start=(cb == 0),
                        stop=(cb == CB - 1),
                    )
                nc.vector.tensor_tensor(
                    out=vv[tb][:, :, 0:HD],
                    in0=pv[:].rearrange("p (a b) -> p a b", a=NH),
                    in1=bvb[:].rearrange("p (a b) -> p a b", a=NH),
                    op=mybir.AluOpType.add,
                )

            # ---------------- attention -------------------------------------
            # kb-outer over PAIRS of 512-wide q chunks (A, B): each kT / vv
            # weight load covers two matmuls (one per chunk), halving the
            # weight-switch overhead.  Per kb: scores for h0 (chunks A,B) and
            # h1 (concurrent on the other row strips), 4 exp ops of [128,512]
            # split across ACT (true exp) / DVE (Schraudolph) by (kb+s)
            # parity, then AV(kb-1) (4 matmuls, weight load shared per head).
            # The 4 score slots are single-bank, reused kb -> kb+1 with
            # per-tile deps; pY accumulators are the 4 Y banks.  The last AV
            # and the 4 evacuation copies are deferred into the next
            # super-iteration's first kbs so they never stall the PE.
            pending = []

            def attention2(pr, qcp):
                qb = qcp * 2  # first 512-chunk index of this pair
                pY = {
                    (c, s): ps.tile(
                        [P, 512], F32, tag=f"Y{2 * c + s}", name=f"pY{c}{s}"
                    )
                    for c in range(2)
                    for s in range(2)
                }

                def issue_av(kb, epair, pY=pY, pr=pr):
                    for s in range(2):
                        for c in range(2):
                            nc.tensor.matmul(
                                pY[(c, s)][:],
                                lhsT=vv[kb][:, 2 * pr + s, :],
                                rhs=epair[c][:, ts(s, 512)],
                                start=(kb == 0),
                                stop=(kb == TB - 1),
                            )

                def make_evac(c, s, pY=pY, pr=pr, qb=qb):
                    def ev():
                        yst = ystage.tile([HD + 1, 512], F32, name="yst")
                        if (c + s) % 2 == 0:
                            nc.scalar.copy(out=yst[:], in_=pY[(c, s)][0 : HD + 1, :])
                        else:
                            nc.vector.tensor_copy(
                                out=yst[:], in_=pY[(c, s)][0 : HD + 1, :]
                            )
                        nc.sync.dma_start(
                            out=y_d[
                                pr * P + s * HD : pr * P + (s + 1) * HD,
                                ts(qb + c, 512),
                            ],
                            in_=yst[0:HD, :],
                        )
                        nc.sync.dma_start(
                            out=den_d[2 * pr + s : 2 * pr + s + 1, ts(qb + c, 512)],
                            in_=yst[HD : HD + 1, :],
                        )

                    return ev

                e_hist = []
                for kb in range(TB):
                    pS = {
                        (c, s): ps.tile(
                            [P, 512], F32, tag=f"S{c}{s}", name=f"pS{c}{s}"
                        )
                        for c in range(2)
                        for s in range(2)
                    }
                    for s in range(2):  # one kT load per head, 2 chunks each
                        for c in range(2):
                            nc.tensor.matmul(
                                pS[(c, s)][:],
                                lhsT=kT[pr][ts(s, 64), ts(kb, P)],
                                rhs=qT[pr][ts(s, 64), ts(qb + c, 512)],
                                start=True,
                                stop=True,
                            )
                    epair = [
                        epool.tile([P, 2 * 512], F16, name=f"eT{c}")
                        for c in range(2)
                    ]
                    for c in range(2):
                        for s in range(2):
                            if (kb + s) % 2 == 0:
                                nc.scalar.activation(
                                    out=epair[c][:, ts(s, 512)],
                                    in_=pS[(c, s)][:],
                                    func=mybir.ActivationFunctionType.Exp,
                                    scale=0.125,
                                )
                            else:
                                nc.vector.tensor_scalar(
                                    out=epair[c][:, ts(s, 512)].bitcast(I16),
                                    in0=pS[(c, s)][:],
                                    scalar1=SCH_MUL,
                                    scalar2=SCH_ADD,
                                    op0=mybir.AluOpType.mult,
                                    op1=mybir.AluOpType.add,
                                )
                    e_hist.append(epair)
                    # deferred work from the previous chunk-pair: the final
                    # AV at kb=0, all four evacuations by kb=1 -- everything
                    # must be issued before AV(0) rewrites the Y banks
                    if pending:
                        if kb == 0:
                            for _ in range(3):
                                if pending:
                                    pending.pop(0)()
                        elif kb == 1:
                            while pending:
                                pending.pop(0)()
                    if kb >= 1:
                        issue_av(kb - 1, e_hist[kb - 1])
                pending.append(
                    lambda eh=e_hist, ia=issue_av: ia(TB - 1, eh[TB - 1])
                )
                for c in range(2):
                    for s in range(2):
                        pending.append(make_evac(c, s))

            # kT pair-0 first (scores need all of k before any q chunk), then
            # q pair-0 and all of v; pair-1 projections between the two
            # attention passes
            qk_proj(2)
            qk_proj(0)
            for tb in range(TB):
                v_proj(tb)
            attention2(0, 0)
            attention2(0, 1)
            while pending:  # flush before proj reuses the Y banks
                pending.pop(0)()
            qk_proj(1)
            qk_proj(3)
            attention2(1, 0)
            attention2(1, 1)
            while pending:
                pending.pop(0)()

    if finalize:
        nc.finalize()
    return nc


def _shard_inputs(x, W_qkv, b_qkv):
    """Build per-core input maps. Core c: batch c//4, head group c%4."""
    x = np.asarray(x, dtype=np.float32)
    W = np.asarray(W_qkv, dtype=np.float32)
    b = np.asarray(b_qkv, dtype=np.float32)
    bf = np.float16
    xT = [np.ascontiguousarray(x[bi].T.astype(bf)) for bi in range(2)]
    in_maps = []
    for c in range(8):
        bi, hg = c // 4, c % 4
        cs = hg * 256  # column start within each of q/k/v blocks
        w_core = np.concatenate(
            [
                W[:, cs : cs + 256],
                W[:, D + cs : D + cs + 256],
                W[:, 2 * D + cs : 2 * D + cs + 256],
            ],
            axis=1,
        ).astype(bf)
        bqk = np.concatenate([b[cs : cs + 256], b[D + cs : D + cs + 256]])
        bqk = np.ascontiguousarray(bqk.reshape(4, 128).T)
        bv = np.ascontiguousarray(b[2 * D + cs : 2 * D + cs + 256].reshape(1, 256))
        in_maps.append(
            {
                "xT": xT[bi],
                "w": np.ascontiguousarray(w_core),
                "bqk": bqk,
                "bv": bv,
            }
        )
    return in_maps


def kernel(x, W_qkv, b_qkv, trace=False):
    from concourse.bass_utils import run_bass_kernel_spmd

    if "nc" not in _CACHED:
        _CACHED["nc"] = build_bass()
    nc = _CACHED["nc"]

    in_maps = _shard_inputs(x, W_qkv, b_qkv)
    res = run_bass_kernel_spmd(nc, in_maps, list(range(8)), trace=trace)
    _CACHED["last_result"] = res

    out = np.empty((2, T, D), dtype=np.float32)
    for c in range(8):
        bi, hg = c // 4, c % 4
        yT = res.results[c]["y"]  # [256, T] unnormalized, head-major
        den = res.results[c]["den"]  # [4, T]
        y = (yT.reshape(NH, HD, T) / den[:, None, :]).transpose(2, 0, 1)
        out[bi, :, hg * 256 : (hg + 1) * 256] = y.reshape(T, NH * HD)
    return out


if __name__ == "__main__":
    nc = build_bass()
    print("built ok")
